# revision 21
# baseline (speedup 1.0000x reference)
"""GNN message-passing layer (ConvolutionLayer) on 8 Trainium2 NeuronCores.

Reference computation (per graph b):
    deg[i]   = sum_j adj[b,i,j]
    agg      = (adj / deg) @ node_mat            # [N, Fin]
    out      = leaky_relu(agg @ W.T + b, 0.01)   # [N, Fout]

Device strategy (pure data parallel over the batch, 8 graphs per core):
  * adj is fed transposed (At[j, i]) so the TensorEngine can contract j
    (its partition dim).
  * MM1: P[i, c] = At_tile.T @ X'_tile where X' = [node_mat | 1].  The
    appended ones-column makes column F of P the row degree, so deg comes
    for free with the matmul.  Inputs are bf16 (halves the dominant DMA
    traffic); PSUM accumulation and everything downstream stay fp32.
  * agg = P[:, :F] * (1/deg): per-partition scalar multiply on DVE, fused
    with the PSUM->SBUF copy.
  * MM2 needs agg^T as the stationary operand: one PE transpose per
    [128,128] tile, then out[i, o] = agg^T.T @ W^T (fp32), bias-add (DVE),
    leaky-relu on ACT (hw Lrelu verified bitwise == max(t, 0.01*t)), DMA out.

All DRAM tensors use host-side partition-blocked layouts so every DMA
moves multi-KB contiguous runs per partition (few descriptors — HWDGE
descriptor processing otherwise dominates):
  at_in [128, BPC, NT, N]   : at_in[p, g, jt, i] = adj[g, i, jt*128+p)
  x_in  [128, BPC*NT, F+1]  : x_in[p, g*NT+jt, c] = node_mat[g, jt*128+p, c],
                              with column F == 1.0
  o_out [128, BPC, NT, F]   : o_out[p, g, it, o] = out[g, it*128+p, o]
"""

import numpy as np
import ml_dtypes

import concourse.mybir as mybir
import concourse.tile as tile
from concourse import bacc
from concourse.bass_utils import run_bass_kernel_spmd
from concourse.masks import make_identity

N_CORES = 8
B, N, F = 64, 1024, 128
BPC = B // N_CORES          # graphs per core
NT = N // 128               # 128-row tiles per graph
LEAKY_SLOPE = 0.01
# Lrelu on ACT measured bitwise-identical to max(t, 0.01*t) on DVE on HW;
# the DVE variant is kept for CoreSim (which lacks Lrelu).
LEAKY_ON_ACT = True

IN_DT = mybir.dt.bfloat16
IN_NP = ml_dtypes.bfloat16
F32 = mybir.dt.float32

_CACHE = {}


def build_nc(repeat=None):
    """Build + compile the per-core kernel. `repeat` (benchmark only) wraps
    the whole body in a hardware For_i loop so device time can be measured
    as a slope over repeat counts, amortizing dispatch/tunnel overhead."""
    nc = bacc.Bacc(
        "TRN2", target_bir_lowering=False, debug=False, num_devices=N_CORES
    )
    at_d = nc.dram_tensor(
        "at_in", [128, BPC, NT, N], IN_DT, kind="ExternalInput"
    ).ap()
    x_d = nc.dram_tensor(
        "x_in", [128, BPC * NT, F + 1], IN_DT, kind="ExternalInput"
    ).ap()
    wt_d = nc.dram_tensor("wt_in", [F, F], F32, kind="ExternalInput").ap()
    bb_d = nc.dram_tensor("bb_in", [128, F], F32, kind="ExternalInput").ap()
    o_d = nc.dram_tensor(
        "o_out", [128, BPC, NT, F], F32, kind="ExternalOutput"
    ).ap()

    with tile.TileContext(nc) as tc:
        with (
            tc.tile_pool(name="consts", bufs=1) as consts,
            tc.tile_pool(name="xp", bufs=1) as xp,
            tc.tile_pool(name="atp", bufs=6) as atp,
            tc.tile_pool(name="work", bufs=6) as work,
            tc.tile_pool(name="obig", bufs=4) as obig,
            tc.tile_pool(name="psp", bufs=4, space="PSUM") as psp,
            tc.tile_pool(name="pst", bufs=2, space="PSUM") as pst,
            tc.tile_pool(name="pso", bufs=2, space="PSUM") as pso,
        ):
            # consts ride the ACT DGE queue so the sync queue's first entries
            # are graph 0's x/At chunks (PE start gates on those).
            wt_sb = consts.tile([F, F], F32)
            nc.scalar.dma_start(wt_sb[:], wt_d[:])
            bb_sb = consts.tile([128, F], F32)
            nc.scalar.dma_start(bb_sb[:], bb_d[:])
            ident = consts.tile([128, 128], F32)
            make_identity(nc, ident[:])

            NH = NT // 2  # At / output DMAs are split in jt/i halves so the
            # first matmuls (and last stores) overlap the bulk DMA stream.

            def body(_it=None):
                for g in range(BPC):
                    x_g = xp.tile(
                        [128, NT, F + 1], IN_DT, name=f"x_{g}", tag=f"x_{g}"
                    )
                    nc.sync.dma_start(
                        x_g[:], x_d[:, g * NT : (g + 1) * NT, :]
                    )
                    # graph 0's At arrives in quarters so the first matmuls
                    # start ~1.5us after launch; later graphs use halves.
                    n_chunks = 4 if g == 0 else 2
                    csz = NT // n_chunks
                    at_chunks = []
                    for h in range(n_chunks):
                        at_gh = atp.tile(
                            [128, csz, N], IN_DT, name=f"at_{g}_{h}",
                            tag=f"at{csz}",
                        )
                        nc.sync.dma_start(
                            at_gh[:], at_d[:, g, h * csz : (h + 1) * csz]
                        )
                        at_chunks.append(at_gh)

                    o_half = [
                        obig.tile([128, NH, F], F32, name=f"ob_{g}_{h}", tag="ob")
                        for h in range(2)
                    ]

                    for i in range(NT):
                        o_big, io = o_half[i // NH], i % NH
                        p = psp.tile([128, F + 1], F32, name=f"p_{g}_{i}", tag="p")
                        for jt in range(NT):
                            nc.tensor.matmul(
                                p[:],
                                at_chunks[jt // csz][
                                    :, jt % csz, i * 128 : (i + 1) * 128
                                ],
                                x_g[:, jt, :],
                                start=(jt == 0),
                                stop=(jt == NT - 1),
                            )
                        invd = work.tile(
                            [128, 1], F32, name=f"invd_{g}_{i}", tag="invd"
                        )
                        nc.vector.reciprocal(invd[:], p[:, F : F + 1])
                        agg = work.tile(
                            [128, F], F32, name=f"agg_{g}_{i}", tag="agg"
                        )
                        nc.vector.tensor_scalar_mul(agg[:], p[:, 0:F], invd[:])

                        pt = pst.tile([128, 128], F32, name=f"pt_{g}_{i}", tag="pt")
                        nc.tensor.transpose(pt[:], agg[:], ident[:])
                        aggt = work.tile(
                            [128, 128], F32, name=f"aggt_{g}_{i}", tag="aggt"
                        )
                        nc.scalar.copy(aggt[:], pt[:])

                        po = pso.tile([128, F], F32, name=f"po_{g}_{i}", tag="po")
                        nc.tensor.matmul(
                            po[:], aggt[:], wt_sb[:], start=True, stop=True
                        )

                        t = work.tile([128, F], F32, name=f"t_{g}_{i}", tag="t")
                        nc.vector.tensor_add(out=t[:], in0=po[:], in1=bb_sb[:])
                        if LEAKY_ON_ACT:
                            # single fused leaky-relu on the scalar engine
                            nc.scalar.activation(
                                o_big[:, io, :],
                                t[:],
                                mybir.ActivationFunctionType.Lrelu,
                                alpha=LEAKY_SLOPE,
                            )
                        else:
                            # leaky_relu(t) == max(t, 0.01*t), exact fp32
                            u = work.tile([128, F], F32, name=f"u_{g}_{i}", tag="u")
                            nc.scalar.activation(
                                u[:],
                                t[:],
                                mybir.ActivationFunctionType.Copy,
                                scale=LEAKY_SLOPE,
                            )
                            nc.vector.tensor_max(
                                out=o_big[:, io, :], in0=t[:], in1=u[:]
                            )
                        if io == NH - 1:
                            # different DGE queue than the input stream, so
                            # output stores never block input prefetch (FIFO).
                            nc.scalar.dma_start(
                                o_d[:, g, (i // NH) * NH : (i // NH + 1) * NH],
                                o_big[:],
                            )

            if repeat is None:
                body()
            else:
                with tc.For_i(0, repeat, 1) as it:
                    body(it)

    nc.compile()
    return nc


def get_nc():
    if "nc" not in _CACHE:
        _CACHE["nc"] = build_nc()
    return _CACHE["nc"]


def _block_adj(adj_core):
    """[BPC, N(i), N(j)] f32 -> [128(p), BPC, NT, N(i)] bf16 where
    out[p, g, jt, i] = adj[g, i, jt*128 + p]."""
    a = adj_core.reshape(BPC, N, NT, 128)          # [g, i, jt, p]
    return a.transpose(3, 0, 2, 1).astype(IN_NP)   # [p, g, jt, i]


def _block_x(x_core):
    """[BPC, N(j), F] f32 -> [128(p), BPC*NT, F+1] bf16 with ones column."""
    xb = np.ones((128, BPC, NT, F + 1), dtype=IN_NP)
    x = x_core.reshape(BPC, NT, 128, F)            # [g, jt, p, f]
    xb[:, :, :, :F] = x.transpose(2, 0, 1, 3).astype(IN_NP)
    return xb.reshape(128, BPC * NT, F + 1)


def _unblock_out(o_core):
    """[128(p), BPC, NT, F] f32 -> [BPC, N, F]."""
    return o_core.transpose(1, 2, 0, 3).reshape(BPC, N, F)


def make_in_maps(node_mat, adj_mat, W, b):
    wt = np.ascontiguousarray(W.T.astype(np.float32))   # [Fin, Fout]
    bb = np.ascontiguousarray(
        np.broadcast_to(b.astype(np.float32)[None, :], (128, F))
    )
    in_maps = []
    for c in range(N_CORES):
        sl = slice(c * BPC, (c + 1) * BPC)
        in_maps.append(
            {
                "at_in": _block_adj(adj_mat[sl]),
                "x_in": _block_x(node_mat[sl]),
                "wt_in": wt,
                "bb_in": bb,
            }
        )
    return in_maps


def kernel(node_mat, adj_mat, W, b):
    node_mat = np.asarray(node_mat)
    adj_mat = np.asarray(adj_mat)
    W = np.asarray(W)
    b = np.asarray(b)
    nc = get_nc()
    in_maps = make_in_maps(node_mat, adj_mat, W, b)
    res = run_bass_kernel_spmd(nc, in_maps, core_ids=list(range(N_CORES)))
    out = np.concatenate(
        [_unblock_out(r["o_out"]) for r in res.results], axis=0
    )
    return np.ascontiguousarray(out).astype(np.float32)
